# revision 1
# baseline (speedup 1.0000x reference)
"""Trainium2 Bass kernel for nn_ExtractorMLP (GNN edge cosine-similarity).

Math:  out[e] = cos_sim(mlp(emb[col[e]]), mlp(emb[row[e]]))
where  mlp(x) = elu(x @ W1.T + b1) @ W2.T + b2   (b1 = b2 = 0 here)

v6 strategy (edge-data-parallel across 8 cores, one SPMD program):
  * Phase 1 (per node, replicated): UNnormalized MLP table t[v] (bf16),
    written to DRAM in node-major [n_pad, 128] layout (source for both
    the row-side gathers and the col-side block loads).  Per-node
    sum-of-squares ships to the host, which applies the cosine
    normalization at the end.
  * Phase 2 (edges, sharded): per core, edges are bucketed by row-half
    (int16 gather-index limit) and col-sorted within each bucket.  Each
    bucket is packed by a ring schedule: a block sequence L advancing
    ADV entries per 384-edge chunk through a 16-tile ring of 128-node
    table blocks (lo: ADV=2/window 4, hi: ADV=4/window 6 — hi is ~2x
    sparser per block).  Ring tiles are loaded with dynamic-offset DMAs
    (reg_load + snap + bass.ds) from per-core offsets shipped as data,
    alternating between the sync and scalar engines.
      - col side (NO gather): f1 = sum_s block_s.T @ onehot_s, where the
        one-hot [128, 384] slices ship precomputed from the host in fp8
        (exact 0/1) and feed the matmul rhs directly.
      - row side: SWDGE dma_gather (transpose, HBM source), 7680 edges
        per instruction; every core trims at a shared count (0-pad to
        the shared count, -1 beyond), so descriptor-ring bookkeeping
        stays consistent and tail pads cost nothing.
      - dot: DVE multiply f1(PSUM) * f2(SBUF), then contraction over
        the 128 feature partitions with the sliding one-hot matmul
        trick, 128 chunks per PSUM output group.
  * Host: out[e] = dots[e] / (n[col] * n[row]), n = max(sqrt(ssq), eps).
"""

import math

import numpy as np
import ml_dtypes

BF16 = ml_dtypes.bfloat16
FP8 = ml_dtypes.float8_e4m3

H = 128            # feature dim
P = 128            # partitions
CHUNK = 384        # edges per expansion chunk
BATCH = 7680       # edges per dma_gather instruction / f2 tile (20 chunks)
CPB = BATCH // CHUNK
HALF = 32768       # int16 index limit: row-bucket split
NCORES = 8
ST_W = 512         # phase-1 supertile width (nodes)
PAD_CODE = 1000.0  # code value matching no slot
RING = 16          # ring tiles
GEO = {"lo": (2, 4), "hi": (4, 6)}   # region -> (ADV, WINDOW)
W_MAX = 6

_PROG_CACHE: dict = {}
LAST_RESULTS = None


# --------------------------------------------------------------------------
# host-side packing
# --------------------------------------------------------------------------

def _wrap_idx(idx):
    """[S*16] int16 -> [128, S] wrapped layout (16 partitions, replicated 8x)."""
    w = idx.reshape(-1, 16).T.astype(np.int16)
    return np.tile(w, (8, 1))


def _pack_region(cols, rows, row_off, adv, win):
    """Ring-schedule packer for one row-half bucket (col-sorted).

    Builds block sequence L advancing `adv` entries per chunk; chunk c
    draws edges from window L[adv*c : adv*c+win] (slot = window offset).
    Blocks overflowing the expiring window part get repeat entries.
    Returns (codes f32[S], ridx int64[S], pos int64[S], L_blocks).
    """
    n = len(cols)
    blk = (cols >> 7).astype(np.int64)
    bounds = np.flatnonzero(np.diff(blk)) + 1
    starts = np.concatenate([[0], bounds]) if n else np.zeros(0, np.int64)
    ends = np.concatenate([bounds, [n]]) if n else np.zeros(0, np.int64)
    groups = [(int(blk[a]), int(a), int(b)) for a, b in zip(starts, ends)]
    gqi = 0
    L = []
    codes, ridx, pos = [], [], []
    c = 0
    while n:
        while len(L) < adv * c + win:
            if gqi < len(groups):
                L.append(list(groups[gqi]))
                gqi += 1
            else:
                L.append([0, 0, 0])
        c_codes = np.full(CHUNK, PAD_CODE, dtype=np.float32)
        c_ridx = np.zeros(CHUNK, dtype=np.int64)
        c_pos = np.full(CHUNK, -1, dtype=np.int64)
        j = 0
        for s in range(win):
            ent = L[adv * c + s]
            take = min(CHUNK - j, ent[2] - ent[1])
            if take > 0:
                sel = slice(ent[1], ent[1] + take)
                c_codes[j:j + take] = 128 * s + (cols[sel] & 127)
                c_ridx[j:j + take] = rows[sel] - row_off
                c_pos[j:j + take] = np.arange(ent[1], ent[1] + take)
                ent[1] += take
                j += take
        ins = []
        for s in range(adv):
            ent = L[adv * c + s]
            if ent[2] > ent[1]:
                ins.append([ent[0], ent[1], ent[2]])
                ent[1] = ent[2]
        if ins:
            L[adv * c + adv:adv * c + adv] = ins
        codes.append(c_codes)
        ridx.append(c_ridx)
        pos.append(c_pos)
        c += 1
        if gqi >= len(groups) and all(e[2] <= e[1] for e in L[adv * c:]):
            break
    while len(L) < adv * c + win:
        L.append([0, 0, 0])
    L_blocks = [e[0] for e in L]
    if not codes:
        return (np.zeros(0, np.float32), np.zeros(0, np.int64),
                np.zeros(0, np.int64), L_blocks)
    return (np.concatenate(codes), np.concatenate(ridx),
            np.concatenate(pos), L_blocks)


def _prep_core(col, row):
    lo_sel = row < HALF
    out = {}
    for name, sel, roff in (("lo", lo_sel, 0), ("hi", ~lo_sel, HALF)):
        c, r = col[sel], row[sel]
        orig = np.nonzero(sel)[0]
        order = np.argsort(c, kind="stable")
        c, r, orig = c[order], r[order], orig[order]
        adv, win = GEO[name]
        codes, ridx, pos, lblocks = _pack_region(c, r, roff, adv, win)
        opos = np.where(pos >= 0, orig[np.clip(pos, 0, None)], -1)
        out[name] = (codes, ridx, lblocks, opos)
    return out


def _pad_region(codes, ridx, lblocks, opos, cap, adv, shared_real=None):
    """Pad one region's streams to cap edges; L to adv * (cap/CHUNK)."""
    s = len(codes)
    nck = cap // CHUNK
    assert s <= cap and cap % CHUNK == 0
    assert len(codes) // CHUNK < nck, "need >=1 empty tail chunk"
    pc = np.full(cap, PAD_CODE, dtype=np.float32)
    pr = np.full(cap, -1, dtype=np.int64)
    po = np.full(cap, -1, dtype=np.int64)
    pm = np.zeros(nck * adv, dtype=np.uint32)
    pc[:s] = codes
    pr[:s] = ridx
    po[:s] = opos
    lb = (np.array(lblocks[:nck * adv], dtype=np.uint32) * 128)
    pm[:len(lb)] = lb
    if shared_real is not None:
        for b0 in range(0, cap, BATCH):
            cnt = shared_real.get(b0, 0)
            lo = max(s - b0, 0)
            if cnt > lo:
                pr[b0 + lo:b0 + cnt] = 0
    return pc, pr, pm, po


def _geometry(n_lo, n_hi):
    """Per-chunk compile-time schedule shared by program and emulator.

    Returns list over chunks of dicts: region, adv, win, ebase (entry
    index of window start), ohoff (one-hot slice offset in CHUNK units),
    plus total entry and one-hot slot counts.
    """
    chunks = []
    e = 0
    o = 0
    for c in range(n_lo + n_hi):
        name = "lo" if c < n_lo else "hi"
        adv, win = GEO[name]
        chunks.append({"name": name, "adv": adv, "win": win,
                       "ebase": e, "ohoff": o})
        e += adv
        o += win
    return chunks, e, o


def _load_schedule(chunks, n_entries):
    """loads_by_chunk[c] = entry indices to load after chunk c's matmuls;
    preamble = entries loaded before the loop."""
    first_use = {}
    last_use = {}
    for c, g in enumerate(chunks):
        for s in range(g["win"]):
            k = g["ebase"] + s
            first_use.setdefault(k, c)
            last_use[k] = c
    n_used = max(first_use) + 1
    preamble = []
    loads = [[] for _ in chunks]
    for k in range(n_used):
        if k - RING < 0:
            preamble.append(k)
            continue
        at = max(last_use.get(k - RING, -1) + 1, 0)
        assert at <= first_use[k] - 1, (k, at, first_use[k])
        loads[at].append(k)
    return preamble, loads


# --------------------------------------------------------------------------
# device program
# --------------------------------------------------------------------------

def _build_program(n_pad, n_lo, n_hi, gathers, mbounds=None):
    import concourse.bacc as bacc
    import concourse.bass as bass
    import concourse.mybir as mybir
    import concourse.tile as tile
    from concourse import library_config
    from contextlib import ExitStack

    f32 = mybir.dt.float32
    bf16 = mybir.dt.bfloat16
    fp8 = mybir.dt.float8e4
    i16 = mybir.dt.int16
    u32 = mybir.dt.uint32
    Alu = mybir.AluOpType
    Act = mybir.ActivationFunctionType

    n_chunks = n_lo + n_hi
    S = n_chunks * CHUNK
    n_groups = math.ceil(n_chunks / P)
    n_blocks = n_pad // H
    half = min(HALF, n_pad)
    n_batches = S // BATCH

    chunks, n_entries, n_ohslots = _geometry(n_lo, n_hi)
    preamble, loads = _load_schedule(chunks, n_entries)
    n_meta = n_entries + RING

    nc = bacc.Bacc("TRN2", target_bir_lowering=False, debug=False,
                   num_devices=NCORES, num_swdge_queues=1)

    embT = nc.dram_tensor("embT", [P, n_pad], bf16, kind="ExternalInput")
    w1t_d = nc.dram_tensor("w1t", [H, H], bf16, kind="ExternalInput")
    w2t_d = nc.dram_tensor("w2t", [H, H], bf16, kind="ExternalInput")
    oh_d = nc.dram_tensor("oh", [P, n_ohslots * CHUNK], fp8,
                          kind="ExternalInput")
    ridx_d = nc.dram_tensor("ridx", [P, S // 16], i16, kind="ExternalInput")
    meta_d = nc.dram_tensor("meta", [1, n_meta], u32, kind="ExternalInput")
    tbl_d = nc.dram_tensor("tbl_dram", [n_pad, H], bf16, kind="ExternalOutput")
    ssq_d = nc.dram_tensor("ssq", [P, n_blocks], f32, kind="ExternalOutput")
    out_d = nc.dram_tensor("out", [n_groups, P, CHUNK], f32,
                           kind="ExternalOutput")

    g_by_batch = [[] for _ in range(n_batches)]
    for (pos0, n_reg, hf) in gathers:
        bt = pos0 // BATCH
        assert pos0 % BATCH == 0 and 0 < n_reg <= BATCH
        g_by_batch[bt].append((pos0, n_reg, hf))

    with ExitStack() as ctx:
        tc = ctx.enter_context(tile.TileContext(nc))
        const = ctx.enter_context(tc.tile_pool(name="const", bufs=1))
        p1 = ctx.enter_context(tc.tile_pool(name="p1", bufs=3))
        pohd = ctx.enter_context(tc.tile_pool(name="pohd", bufs=8))
        pridx = ctx.enter_context(tc.tile_pool(name="pridx", bufs=4))
        pring = ctx.enter_context(tc.tile_pool(name="pring", bufs=1))
        pf2 = ctx.enter_context(tc.tile_pool(name="pf2", bufs=5))
        pprod = ctx.enter_context(tc.tile_pool(name="pprod", bufs=6))
        psA = ctx.enter_context(tc.tile_pool(name="psA", bufs=4, space="PSUM"))
        psB = ctx.enter_context(tc.tile_pool(name="psB", bufs=4, space="PSUM"))

        nc.gpsimd.load_library(library_config.mlp)

        # --- constants / persistent tiles ---
        w1t = const.tile([H, H], bf16, tag="w1t")
        w2t = const.tile([H, H], bf16, tag="w2t")
        onehot = const.tile([P, 2 * P - 1], bf16, tag="onehot")
        ss_all = const.tile([P, n_blocks], f32, tag="ss_all")
        meta = const.tile([1, n_meta], u32, tag="meta")
        nc.sync.dma_start(out=w1t[:], in_=w1t_d[:])
        nc.sync.dma_start(out=w2t[:], in_=w2t_d[:])
        nc.sync.dma_start(out=meta[:], in_=meta_d[:])
        nc.vector.memset(onehot[:], 0.0)
        nc.vector.memset(onehot[:, P - 1:P], 1.0)

        # --- phase 1: MLP table (unnormalized) + sumsq -> DRAM ---
        n0 = 0
        st = 0
        while n0 < n_pad:
            w = min(ST_W, n_pad - n0)
            nb = w // H
            xt = p1.tile([P, ST_W], bf16, tag="xt", name="xt")[:, :w]
            nc.sync.dma_start(out=xt, in_=embT[:, n0:n0 + w])
            ph1 = psA.tile([P, ST_W], f32, tag="a", name="ph1")[:, :w]
            nc.tensor.matmul(ph1, lhsT=w1t[:], rhs=xt, start=True, stop=True)
            # elu(x) = max(exp(min(x, 0)) - 1, x)
            u_t = p1.tile([P, ST_W], bf16, tag="u", name="u")[:, :w]
            nc.scalar.activation(u_t, ph1, Act.Relu, scale=-1.0)
            e_t = p1.tile([P, ST_W], bf16, tag="e", name="e")[:, :w]
            nc.scalar.activation(e_t, u_t, Act.Exp, scale=-1.0)
            h1_t = p1.tile([P, ST_W], bf16, tag="h1", name="h1")[:, :w]
            nc.vector.scalar_tensor_tensor(
                h1_t, in0=e_t, scalar=-1.0, in1=ph1,
                op0=Alu.add, op1=Alu.max)
            pg = psB.tile([P, ST_W], f32, tag="b", name="pg")[:, :w]
            for b in range(nb):
                nc.tensor.matmul(pg[:, b * H:(b + 1) * H],
                                 lhsT=h1_t[:, b * H:(b + 1) * H],
                                 rhs=w2t[:], start=True, stop=True)
            tb_t = p1.tile([P, ST_W], bf16, tag="tb", name="tb")[:, :w]
            nc.scalar.activation(tb_t, pg, Act.Copy)
            sq_t = p1.tile([P, ST_W], bf16, tag="sq", name="sq")[:, :w]
            for b in range(nb):
                nc.vector.scalar_tensor_tensor(
                    sq_t[:, b * H:(b + 1) * H],
                    in0=pg[:, b * H:(b + 1) * H], scalar=0.0,
                    in1=tb_t[:, b * H:(b + 1) * H],
                    op0=Alu.add, op1=Alu.mult,
                    accum_out=ss_all[:, st * (ST_W // H) + b:
                                     st * (ST_W // H) + b + 1])
            nc.sync.dma_start(
                out=tbl_d[n0:n0 + w, :].rearrange("(s p) f -> p s f", p=P),
                in_=tb_t.rearrange("p (s f) -> p s f", f=H))
            n0 += w
            st += 1
        nc.sync.dma_start(out=ssq_d[:], in_=ss_all[:])

        # --- phase 2 ---
        halves = (tbl_d[0:half, :], tbl_d[half:n_pad, :])

        ring_tiles = [pring.tile([P, H], bf16, tag=f"ring{r}", name=f"ring{r}")
                      for r in range(RING)]

        def _ring_load(k):
            eng = nc.scalar if k % 2 == 0 else nc.sync
            reg = eng.alloc_register(f"roff_{k}")
            eng.reg_load(reg, meta[0:1, k:k + 1])
            mn, mx = (0, n_pad - P) if mbounds is None else mbounds[k]
            off = eng.snap(reg, donate=True, min_val=mn, max_val=mx)
            eng.dma_start(out=ring_tiles[k % RING][:],
                          in_=tbl_d[bass.ds(off, P), :])

        for k in preamble:
            _ring_load(k)

        f2_tiles = {}
        pout = None
        for c in range(n_chunks):
            g = chunks[c]
            if c % CPB == 0:
                bt = c // CPB
                f2t = pf2.tile([P, BATCH], bf16, tag="f2", name=f"f2_{bt}")
                f2_tiles[bt] = f2t
                for (pos0, n_reg, hf) in g_by_batch[bt]:
                    rxt = pridx.tile([P, BATCH // 16], i16, tag="rx",
                                     name=f"rx{bt}")
                    nc.sync.dma_start(
                        out=rxt[:],
                        in_=ridx_d[:, pos0 // 16:(pos0 + BATCH) // 16])
                    f2g = f2t[:, 0:BATCH].rearrange("p (a t) -> p a t", a=1)
                    nc.gpsimd.dma_gather(
                        f2g, halves[hf], rxt[:],
                        BATCH, n_reg, H,
                        transpose=True, single_packet=False,
                        queue_num=0)

            oht = pohd.tile([P, W_MAX * CHUNK], fp8, tag="ohd",
                            name=f"oh{c}")[:, :g["win"] * CHUNK]
            nc.scalar.dma_start(
                out=oht,
                in_=oh_d[:, g["ohoff"] * CHUNK:
                         (g["ohoff"] + g["win"]) * CHUNK])
            f1p = psA.tile([P, ST_W], f32, tag="a", name=f"f1_{c}")[:, :CHUNK]
            for s in range(g["win"]):
                nc.tensor.matmul(
                    f1p[:], lhsT=ring_tiles[(g["ebase"] + s) % RING][:],
                    rhs=oht[:, s * CHUNK:(s + 1) * CHUNK],
                    start=(s == 0), stop=(s == g["win"] - 1))
            for k in loads[c]:
                _ring_load(k)
            bt, off_b = divmod(c * CHUNK, BATCH)
            prod = pprod.tile([P, CHUNK], bf16, tag="prod", name=f"pr{c}")
            nc.vector.tensor_tensor(out=prod[:], in0=f1p[:],
                                    in1=f2_tiles[bt][:, off_b:off_b + CHUNK],
                                    op=Alu.mult)
            gq, p = divmod(c, P)
            if p == 0:
                pout = psB.tile([P, ST_W], f32, tag="b",
                                name=f"po{gq}")[:, :CHUNK]
            last = c == n_chunks - 1
            nc.tensor.matmul(pout[:], lhsT=onehot[:, P - 1 - p:2 * P - 1 - p],
                             rhs=prod[:], start=(p == 0),
                             stop=(p == P - 1 or last))
            if p == P - 1 or last:
                rows = p + 1
                ost = p1.tile([P, CHUNK], f32, tag="ost",
                              name=f"ost{gq}")[:rows]
                nc.vector.tensor_copy(out=ost, in_=pout[:rows])
                nc.sync.dma_start(out=out_d[gq, :rows], in_=ost)

    nc.compile()
    return nc


# --------------------------------------------------------------------------
# numpy emulation of phase 2 (host-side self-test)
# --------------------------------------------------------------------------

def _emulate_core(table_f32, codes, ridx, meta, half_starts, n_lo, n_hi):
    chunks, _, _ = _geometry(n_lo, n_hi)
    n_chunks = n_lo + n_hi
    S = n_chunks * CHUNK
    iota = np.arange(P)[:, None] + 128 * np.arange(W_MAX)[None, :]
    dots = np.zeros(S, dtype=np.float32)
    for c in range(n_chunks):
        g = chunks[c]
        cd = codes[c * CHUNK:(c + 1) * CHUNK]
        f1 = np.zeros((H, CHUNK), dtype=np.float32)
        for s in range(g["win"]):
            oh = (cd[None, :] == iota[:, s:s + 1]).astype(np.float32)
            r0 = int(meta[g["ebase"] + s])
            f1 += table_f32[r0:r0 + P, :].T @ oh
        ri = ridx[c * CHUNK:(c + 1) * CHUNK]
        base = half_starts[c]
        idx = np.where(ri >= 0, ri + base, 0)
        f2 = table_f32[idx, :].T
        prod = (f1 * f2).astype(BF16).astype(np.float32)
        prod[:, ri < 0] = 0.0
        dots[c * CHUNK:(c + 1) * CHUNK] = prod.sum(axis=0)
    return dots


# --------------------------------------------------------------------------
# entry point
# --------------------------------------------------------------------------

def _ensure_ntff_hook():
    import sys
    import types
    try:
        import antenv.axon_hooks  # noqa: F401
        return
    except ImportError:
        pass
    try:
        import antenv
        from trn_agent_boot.trn_boot import _ntff_profile_via_ctypes
        mod = types.ModuleType("antenv.axon_hooks")
        mod._hook = _ntff_profile_via_ctypes("/opt/axon/libaxon_pjrt.so")
        mod.get_axon_ntff_profile_hook = lambda: mod._hook
        mod.set_axon_ntff_profile_hook = lambda h: setattr(mod, "_hook", h)
        sys.modules["antenv.axon_hooks"] = mod
        antenv.axon_hooks = mod
    except Exception:
        pass


def _host_prep(col, row, E):
    ec = E // NCORES
    cores = [_prep_core(col[k * ec:(k + 1) * ec], row[k * ec:(k + 1) * ec])
             for k in range(NCORES)]
    cap = {}
    for name in ("lo", "hi"):
        mx = max(len(cr[name][0]) for cr in cores)
        cap[name] = ((mx // BATCH) + 1) * BATCH
    S = cap["lo"] + cap["hi"]
    real = {nm: [len(cr[nm][0]) for cr in cores] for nm in ("lo", "hi")}
    gathers = []
    for pos0 in range(0, cap["lo"], BATCH):
        mx = max(min(L - pos0, BATCH) for L in real["lo"])
        n_reg = min(BATCH, ((max(mx, 0) + 15) // 16) * 16)
        if n_reg > 0:
            gathers.append((pos0, n_reg, 0))
    for pos0 in range(cap["lo"], S, BATCH):
        q0 = pos0 - cap["lo"]
        mx = max(min(L - q0, BATCH) for L in real["hi"])
        n_reg = min(BATCH, ((max(mx, 0) + 15) // 16) * 16)
        if n_reg > 0:
            gathers.append((pos0, n_reg, 1))
    return cores, cap, S, gathers


def _assemble_core(cr, cap, shared):
    """Build one core's padded streams. Returns dict + opos."""
    n_lo = cap["lo"] // CHUNK
    n_hi = cap["hi"] // CHUNK
    lc, lr, lm, lp = _pad_region(*cr["lo"], cap["lo"], GEO["lo"][0],
                                 shared["lo"])
    hc, hr, hm, hp = _pad_region(*cr["hi"], cap["hi"], GEO["hi"][0],
                                 shared["hi"])
    codes = np.concatenate([lc, hc])
    ridx = np.concatenate([lr, hr])
    meta = np.concatenate([lm, hm, np.zeros(RING, np.uint32)])
    opos = np.concatenate([lp, hp])
    # fp8 one-hot stream: per chunk `win` slices of [128, CHUNK]
    chunks, _, n_ohslots = _geometry(n_lo, n_hi)
    oh = np.zeros((P, n_ohslots, CHUNK), dtype=FP8)
    cd_all = codes.reshape(-1, CHUNK).astype(np.int32)
    for c, g in enumerate(chunks):
        cd = cd_all[c]
        v = cd < 128 * g["win"]
        e = np.arange(CHUNK)[v]
        oh[cd[v] & 127, g["ohoff"] + (cd[v] >> 7), e] = 1
    return codes, ridx, meta, opos, oh.reshape(P, -1)


def kernel(emb, edge_index, W1, b1, W2, b2):
    global LAST_RESULTS
    from concourse.bass_utils import run_bass_kernel_spmd
    _ensure_ntff_hook()

    emb = np.asarray(emb, dtype=np.float32)
    W1 = np.asarray(W1, dtype=np.float32)
    W2 = np.asarray(W2, dtype=np.float32)
    b1 = np.asarray(b1, dtype=np.float32)
    b2 = np.asarray(b2, dtype=np.float32)
    assert np.abs(b1).max() == 0 and np.abs(b2).max() == 0, \
        "nonzero biases not implemented"
    col = np.asarray(edge_index[0]).astype(np.int64)
    row = np.asarray(edge_index[1]).astype(np.int64)

    n, h = emb.shape
    assert h == H
    E = col.shape[0]
    ec = E // NCORES
    n_pad = ((n + P - 1) // P) * P

    cores, cap, S, gathers = _host_prep(col, row, E)
    n_lo = cap["lo"] // CHUNK
    n_hi = cap["hi"] // CHUNK

    shared0 = {"lo": {}, "hi": {}}
    metas = []
    for cr in cores:
        _, _, m, _, _ = None, None, None, None, None
        lm = _pad_region(*cr["lo"], cap["lo"], GEO["lo"][0])[2]
        hm = _pad_region(*cr["hi"], cap["hi"], GEO["hi"][0])[2]
        metas.append(np.concatenate([lm, hm, np.zeros(RING, np.uint32)]))
    marr = np.stack(metas).astype(np.int64)
    mbounds = tuple((int(marr[:, k].min()), int(marr[:, k].max()))
                    for k in range(marr.shape[1]))

    key = (n_pad, n_lo, n_hi, tuple(gathers), mbounds)
    if key not in _PROG_CACHE:
        _PROG_CACHE[key] = _build_program(n_pad, n_lo, n_hi, gathers,
                                          mbounds)
    nc = _PROG_CACHE[key]

    embT = np.zeros((P, n_pad), dtype=BF16)
    embT[:, :n] = emb.T.astype(BF16)
    w1t = W1.T.astype(BF16)
    w2t = W2.T.astype(BF16)

    shared = {"lo": {}, "hi": {}}
    for (pos0, n_reg, hf) in gathers:
        nm = "lo" if hf == 0 else "hi"
        b0 = pos0 if hf == 0 else pos0 - cap["lo"]
        shared[nm][b0] = n_reg

    in_maps = []
    opos_all = []
    for cr in cores:
        codes, ridx, meta, opos, oh = _assemble_core(cr, cap, shared)
        in_maps.append({
            "embT": embT, "w1t": w1t, "w2t": w2t,
            "oh": oh,
            "ridx": _wrap_idx(ridx),
            "meta": meta.reshape(1, -1).astype(np.uint32),
        })
        opos_all.append(opos)

    res = run_bass_kernel_spmd(nc, in_maps, core_ids=list(range(NCORES)))
    LAST_RESULTS = res

    out = np.empty(E, dtype=np.float32)
    for k in range(NCORES):
        r = res.results[k]
        ssq = np.asarray(r["ssq"], dtype=np.float32)
        nrm = np.maximum(np.sqrt(ssq.T.reshape(-1)[:n]), 1e-8)
        dots = np.asarray(r["out"], dtype=np.float32).reshape(-1)[:S]
        opos = opos_all[k]
        valid = opos >= 0
        seg = out[k * ec:(k + 1) * ec]
        seg[opos[valid]] = dots[valid]
        cseg = col[k * ec:(k + 1) * ec]
        rseg = row[k * ec:(k + 1) * ec]
        seg /= nrm[cseg] * nrm[rseg]
    return out



# revision 5
# speedup vs baseline: 3.6314x; 3.6314x over previous
"""Trainium2 Bass kernel for nn_ExtractorMLP (GNN edge cosine-similarity).

Math:  out[e] = cos_sim(mlp(emb[col[e]]), mlp(emb[row[e]]))
where  mlp(x) = elu(x @ W1.T + b1) @ W2.T + b2   (b1 = b2 = 0 here)

v8 strategy (edge-data-parallel across 8 cores, one SPMD program):
  * Host deals edges so all 8 cores share ONE static schedule: edges are
    bucketed by row-half (int16 gather-index limit), sorted by col-block,
    and each (half, col-block) group is dealt round-robin across the 8
    cores.  Every core holds ceil(n_hb/8) edges of block b -- per-block
    run lengths are IDENTICAL across cores, so the whole phase-2 program
    is compile-time static (no ring/window machinery, no dynamic DMAs).
  * Phase 1 (per node, replicated): unnormalized MLP table written BOTH
    to a persistent SBUF tile (node-major blocks; matmul rhs source) and
    to DRAM (gather source + host-side norm computation).
  * Phase 2 (edges, sharded), EDGE-MAJOR layout (partition = edge):
      - row side (f2): SWDGE dma_gather WITHOUT transpose (edge j ->
        partition j%128, its 256B feature row along the free axis),
        4096 edges per instruction, round-robined over all 4 SWDGE
        queues.  Non-transpose gathers do not touch the shared xbar, so
        concurrent queues are safe (transpose gathers are NOT).
      - col side (f1): per 128-edge chunk, variable-length run matmuls:
        lhsT = fp8 one-hot slice [128 nodes x len] (the edges of one
        col-block), rhs = SBUF table block [128 nodes x 128 feats],
        out = PSUM [len edges x 128 feats] at the chunk's partition
        offset.
      - dot: DVE multiply f1(PSUM) * f2(SBUF) per 512-slot PSUM bank,
        then a segmented tensor_reduce (axis=X) -> [128, 4] dots.
  * Host: out[e] = dots[e] / (n[col] * n[row]), n = max(sqrt(ssq), eps),
    ssq computed on host from the returned bf16 table.
"""

import math

import numpy as np
import ml_dtypes

BF16 = ml_dtypes.bfloat16
FP8 = ml_dtypes.float8_e4m3

H = 128            # feature dim
P = 128            # partitions
CHUNK = 128        # edges per PSUM chunk (partition dim)
BANK = 512         # PSUM bank slots (4 chunks) = DVE op granularity
BATCH = 4096       # edges per dma_gather instruction / f2 tile (8 banks)
HALF = 32768       # int16 index limit: row-bucket split
NCORES = 8
ST_W = 512         # phase-1 supertile width (nodes)
NQ = 4             # SWDGE queues for gathers

_PROG_CACHE: dict = {}
LAST_RESULTS = None


# --------------------------------------------------------------------------
# host-side dealing / packing
# --------------------------------------------------------------------------

def _deal(col, row):
    """Deal edges to cores so every core shares the same run schedule.

    Returns:
      sched: per bucket h, list of (block, s_hb) in block order
      order: edge permutation (sorted by (half, block))
      core_of, slot_of: per sorted edge, its core and bucket-local slot
      caps: [2] padded bucket capacities (multiples of BATCH)
      R: [2] shared per-core slot counts (real + intra-block pads)
    """
    E = col.shape[0]
    hb = (row >= HALF).astype(np.int64) * 512 + (col >> 7)
    order = np.argsort(hb, kind="stable")
    hb_s = hb[order]
    bounds = np.flatnonzero(np.diff(hb_s)) + 1
    starts = np.concatenate([[0], bounds])
    ends = np.concatenate([bounds, [E]])
    gkeys = hb_s[starts]
    sizes = ends - starts

    s_g = (sizes + NCORES - 1) // NCORES          # shared slots per group
    ghalf = gkeys >> 9
    base = np.zeros(len(gkeys), dtype=np.int64)
    R = [0, 0]
    for h in (0, 1):
        m = ghalf == h
        cs = np.concatenate([[0], np.cumsum(s_g[m])])
        base[m] = cs[:-1]
        R[h] = int(cs[-1])

    j = np.arange(E) - np.repeat(starts, sizes)
    rot = np.zeros(len(gkeys), dtype=np.int64)
    r = 0
    rem = sizes % NCORES
    for gi in range(len(gkeys)):
        rot[gi] = r
        r = (r + int(rem[gi])) % NCORES
    rot_e = np.repeat(rot, sizes)
    core_of = ((j + rot_e) % NCORES).astype(np.int64)
    slot_of = np.repeat(base, sizes) + j // NCORES

    sched = [[], []]
    for gi in range(len(gkeys)):
        sched[int(ghalf[gi])].append((int(gkeys[gi] & 511), int(s_g[gi])))

    caps = [((R[h] + BATCH - 1) // BATCH) * BATCH if R[h] else 0
            for h in (0, 1)]
    return sched, order, core_of, slot_of, caps, R


def _chunk_schedule(sched_h, R_h, cap_h):
    """Split a bucket's run list into 128-edge chunks.

    chunks[c] = [(block, part_off, len)]; chunks past the live count
    have no runs (skipped entirely -- their dots are garbage the host
    ignores).  Each run becomes one full-width accumulate matmul whose
    lhsT is a per-run masked one-hot (PE output partition offsets must
    be quadrant-aligned, so runs cannot slice the partition dim).
    """
    n_chunks = cap_h // CHUNK
    n_live = (R_h + CHUNK - 1) // CHUNK
    chunks = [[] for _ in range(n_chunks)]
    pos = 0
    for (b, s) in sched_h:
        while s > 0:
            c, off = divmod(pos, CHUNK)
            take = min(s, CHUNK - off)
            chunks[c].append((b, off, take))
            pos += take
            s -= take
    return chunks, n_live


# --------------------------------------------------------------------------
# device program
# --------------------------------------------------------------------------

def _run_layout(caps, chunk_scheds, n_lives):
    """Global run numbering in chunk order + per-batch run ranges.

    Returns (runs_by_chunk, batch_runs, n_runs) where runs_by_chunk[c] =
    [(block, run_gid)], batch_runs[bt] = (r0, r1).
    """
    S = caps[0] + caps[1]
    n_chunks = S // CHUNK
    n_lo_chunks = caps[0] // CHUNK
    runs_by_chunk = []
    gid = 0
    for c in range(n_chunks):
        h, lc = (0, c) if c < n_lo_chunks else (1, c - n_lo_chunks)
        rs = []
        if lc < n_lives[h]:
            for (b, off, ln) in chunk_scheds[h][lc]:
                rs.append((b, off, ln, gid))
                gid += 1
        runs_by_chunk.append(rs)
    n_runs = gid
    cpb = BATCH // CHUNK
    batch_runs = []
    for bt in range(S // BATCH):
        gids = [r[3] for c in range(bt * cpb, (bt + 1) * cpb)
                for r in runs_by_chunk[c]]
        if gids:
            batch_runs.append((min(gids), max(gids) + 1))
        else:
            batch_runs.append((0, 0))
    return runs_by_chunk, batch_runs, n_runs


def _build_program(n_pad, caps, chunk_scheds, n_lives, gathers):
    import concourse.bacc as bacc
    import concourse.mybir as mybir
    import concourse.tile as tile
    from concourse import library_config
    from contextlib import ExitStack

    f32 = mybir.dt.float32
    bf16 = mybir.dt.bfloat16
    fp8 = mybir.dt.float8e4
    i16 = mybir.dt.int16
    Alu = mybir.AluOpType
    Act = mybir.ActivationFunctionType
    Axis = mybir.AxisListType

    S = caps[0] + caps[1]
    n_batches = S // BATCH
    n_chunks = S // CHUNK
    n_blocks = n_pad // H
    half = min(HALF, n_pad)

    runs_by_chunk, batch_runs, n_runs = _run_layout(
        caps, chunk_scheds, n_lives)
    max_batch_runs = max(r1 - r0 for (r0, r1) in batch_runs)

    nc = bacc.Bacc("TRN2", target_bir_lowering=False, debug=False,
                   num_devices=NCORES, num_swdge_queues=NQ)

    embT = nc.dram_tensor("embT", [P, n_pad], bf16, kind="ExternalInput")
    w1t_d = nc.dram_tensor("w1t", [H, H], bf16, kind="ExternalInput")
    w2t_d = nc.dram_tensor("w2t", [H, H], bf16, kind="ExternalInput")
    oh_d = nc.dram_tensor("oh", [P, n_runs * CHUNK], fp8,
                          kind="ExternalInput")
    ridx_d = nc.dram_tensor("ridx", [P, S // 16], i16, kind="ExternalInput")
    tbl_d = nc.dram_tensor("tbl_dram", [n_pad, H], bf16, kind="ExternalOutput")
    out_d = nc.dram_tensor("out", [P, n_chunks], f32, kind="ExternalOutput")

    with ExitStack() as ctx:
        tc = ctx.enter_context(tile.TileContext(nc))
        const = ctx.enter_context(tc.tile_pool(name="const", bufs=1))
        p1 = ctx.enter_context(tc.tile_pool(name="p1", bufs=3))
        poh = ctx.enter_context(tc.tile_pool(name="poh", bufs=2))
        # one rxt tile per batch: SWDGE gathers on different queues can
        # retire out of order, so WAR reuse of idx tiles is unsafe
        pridx = ctx.enter_context(tc.tile_pool(name="pridx",
                                               bufs=max(n_batches, 2)))
        pf2 = ctx.enter_context(tc.tile_pool(name="pf2", bufs=5))
        pprod = ctx.enter_context(tc.tile_pool(name="pprod", bufs=4))
        psA = ctx.enter_context(tc.tile_pool(name="psA", bufs=4, space="PSUM"))
        psB = ctx.enter_context(tc.tile_pool(name="psB", bufs=4, space="PSUM"))

        nc.gpsimd.load_library(library_config.mlp)

        # --- constants / persistent tiles ---
        w1t = const.tile([H, H], bf16, tag="w1t")
        w2t = const.tile([H, H], bf16, tag="w2t")
        tblsb = const.tile([P, n_blocks * H], bf16, tag="tblsb")
        dots = const.tile([P, n_chunks], f32, tag="dots")
        nc.sync.dma_start(out=w1t[:], in_=w1t_d[:])
        nc.sync.dma_start(out=w2t[:], in_=w2t_d[:])

        # --- phase 1: MLP table (unnormalized) -> SBUF + DRAM ---
        n0 = 0
        st = 0
        while n0 < n_pad:
            w = min(ST_W, n_pad - n0)
            nb = w // H
            xt = p1.tile([P, ST_W], bf16, tag="xt", name="xt")[:, :w]
            nc.sync.dma_start(out=xt, in_=embT[:, n0:n0 + w])
            ph1 = psA.tile([P, ST_W], f32, tag="a", name="ph1")[:, :w]
            nc.tensor.matmul(ph1, lhsT=w1t[:], rhs=xt, start=True, stop=True)
            # elu(x) = max(exp(min(x, 0)) - 1, x)
            u_t = p1.tile([P, ST_W], bf16, tag="u", name="u")[:, :w]
            nc.scalar.activation(u_t, ph1, Act.Relu, scale=-1.0)
            e_t = p1.tile([P, ST_W], bf16, tag="e", name="e")[:, :w]
            nc.scalar.activation(e_t, u_t, Act.Exp, scale=-1.0)
            h1_t = p1.tile([P, ST_W], bf16, tag="h1", name="h1")[:, :w]
            nc.vector.scalar_tensor_tensor(
                h1_t, in0=e_t, scalar=-1.0, in1=ph1,
                op0=Alu.add, op1=Alu.max)
            pg = psB.tile([P, ST_W], f32, tag="b", name="pg")[:, :w]
            for b in range(nb):
                nc.tensor.matmul(pg[:, b * H:(b + 1) * H],
                                 lhsT=h1_t[:, b * H:(b + 1) * H],
                                 rhs=w2t[:], start=True, stop=True)
            tb_t = tblsb[:, n0:n0 + w]
            # alternate the PSUM->SBUF copy between scalar and vector
            if st % 2 == 0:
                nc.scalar.activation(tb_t, pg, Act.Copy)
            else:
                nc.vector.tensor_copy(out=tb_t, in_=pg)
            nc.sync.dma_start(
                out=tbl_d[n0:n0 + w, :].rearrange("(s p) f -> p s f", p=P),
                in_=tb_t.rearrange("p (s f) -> p s f", f=H))
            n0 += w
            st += 1

        # --- phase 2 (edge-major) ---
        halves = (tbl_d[0:half, :], tbl_d[half:n_pad, :])
        n_lo_chunks = caps[0] // CHUNK

        def _loc(c):
            return (0, c) if c < n_lo_chunks else (1, c - n_lo_chunks)

        live = [(_loc(c)[1] < n_lives[_loc(c)[0]]) for c in range(n_chunks)]

        g_by_batch = {}
        for (pos0, n_reg, hf) in gathers:
            assert pos0 % BATCH == 0 and 0 < n_reg <= BATCH
            g_by_batch[pos0 // BATCH] = (pos0, n_reg, hf)

        f2_tiles = {}
        oh_tiles = {}
        oh_base = {}
        qn = 0
        for c0 in range(0, n_chunks, BANK // CHUNK):
            if (c0 * CHUNK) % BATCH == 0:
                bt = c0 * CHUNK // BATCH
                if bt in g_by_batch:
                    (pos0, n_reg, hf) = g_by_batch[bt]
                    f2t = pf2.tile([P, BATCH], bf16, tag="f2",
                                   name=f"f2_{bt}")
                    f2_tiles[bt] = f2t
                    rxt = pridx.tile([P, BATCH // 16], i16, tag="rx",
                                     name=f"rx{bt}")
                    nc.scalar.dma_start(
                        out=rxt[:],
                        in_=ridx_d[:, pos0 // 16:(pos0 + BATCH) // 16])
                    f2g = f2t[:, 0:BATCH].rearrange("p (a t) -> p a t", t=H)
                    nc.gpsimd.dma_gather(
                        f2g, halves[hf], rxt[:],
                        BATCH, n_reg, H,
                        transpose=False, single_packet=False,
                        queue_num=qn % NQ)
                    qn += 1
                    (r0, r1) = batch_runs[bt]
                    if r1 > r0:
                        oht = poh.tile([P, max_batch_runs * CHUNK], fp8,
                                       tag="ohd", name=f"ohb{bt}")
                        oh_tiles[bt] = oht
                        oh_base[bt] = r0
                        nc.scalar.dma_start(
                            out=oht[:, :(r1 - r0) * CHUNK],
                            in_=oh_d[:, r0 * CHUNK:r1 * CHUNK])
            bank_chunks = [c for c in range(c0, c0 + BANK // CHUNK)
                           if live[c]]
            if not bank_chunks:
                continue
            bt = c0 * CHUNK // BATCH
            off_b = c0 * CHUNK % BATCH          # slot offset within batch
            f1p = psA.tile([P, BANK], f32, tag="a", name=f"f1_{c0}")
            for c in bank_chunks:
                q = c - c0
                rs = runs_by_chunk[c]
                for i, (b, off, ln, gid) in enumerate(rs):
                    o0 = (gid - oh_base[bt]) * CHUNK
                    nc.tensor.matmul(
                        f1p[:, q * H:(q + 1) * H],
                        lhsT=oh_tiles[bt][:, o0:o0 + CHUNK],
                        rhs=tblsb[:, b * H:(b + 1) * H],
                        start=(i == 0), stop=(i == len(rs) - 1))
            prod = pprod.tile([P, BANK], bf16, tag="prod", name=f"pr{c0}")
            nc.vector.tensor_tensor(
                out=prod[:], in0=f1p[:],
                in1=f2_tiles[bt][:, off_b:off_b + BANK], op=Alu.mult)
            nc.vector.tensor_reduce(
                out=dots[:, c0:c0 + BANK // CHUNK],
                in_=prod.rearrange("p (q f) -> p q f", f=H),
                axis=Axis.X, op=Alu.add)
        nc.sync.dma_start(out=out_d[:], in_=dots[:])

    nc.compile()
    return nc


# --------------------------------------------------------------------------
# numpy emulation of phase 2 (host-side self-test)
# --------------------------------------------------------------------------

def _emulate_core(table_f32, oh, ridx, caps, chunk_scheds, n_lives, gathers):
    S = caps[0] + caps[1]
    n_chunks = S // CHUNK
    dots = np.zeros(S, dtype=np.float32)
    f2_full = np.zeros((S, H), dtype=np.float32)
    for (pos0, n_reg, hf) in gathers:
        base = hf * HALF
        idx = ridx[pos0:pos0 + n_reg].astype(np.int64)
        idx = np.where(idx >= 0, idx + base, 0)
        f2_full[pos0:pos0 + n_reg, :] = table_f32[idx, :]
    n_lo_chunks = caps[0] // CHUNK
    for c in range(n_chunks):
        h, lc = (0, c) if c < n_lo_chunks else (1, c - n_lo_chunks)
        if lc >= n_lives[h]:
            continue
        f1 = np.zeros((CHUNK, H), dtype=np.float32)
        ohc = oh[:, c * CHUNK:(c + 1) * CHUNK].astype(np.float32)
        for (b, off, ln) in chunk_scheds[h][lc]:
            f1[off:off + ln, :] = (
                ohc[:, off:off + ln].T @ table_f32[b * P:(b + 1) * P, :])
        prod = (f1 * f2_full[c * CHUNK:(c + 1) * CHUNK, :]
                ).astype(BF16).astype(np.float32)
        dots[c * CHUNK:(c + 1) * CHUNK] = prod.sum(axis=1)
    return dots


# --------------------------------------------------------------------------
# entry point
# --------------------------------------------------------------------------

def _ensure_ntff_hook():
    import sys
    import types
    try:
        import antenv.axon_hooks  # noqa: F401
        return
    except ImportError:
        pass
    try:
        import antenv
        from trn_agent_boot.trn_boot import _ntff_profile_via_ctypes
        mod = types.ModuleType("antenv.axon_hooks")
        mod._hook = _ntff_profile_via_ctypes("/opt/axon/libaxon_pjrt.so")
        mod.get_axon_ntff_profile_hook = lambda: mod._hook
        mod.set_axon_ntff_profile_hook = lambda h: setattr(mod, "_hook", h)
        sys.modules["antenv.axon_hooks"] = mod
        antenv.axon_hooks = mod
    except Exception:
        pass


def _wrap_idx(idx):
    """[S] int -> [128, S/16] int16 wrapped (16 partitions, replicated 8x)."""
    w = idx.reshape(-1, 16).T.astype(np.int16)
    return np.tile(w, (8, 1))


def kernel(emb, edge_index, W1, b1, W2, b2):
    global LAST_RESULTS
    from concourse.bass_utils import run_bass_kernel_spmd
    _ensure_ntff_hook()

    emb = np.asarray(emb, dtype=np.float32)
    W1 = np.asarray(W1, dtype=np.float32)
    W2 = np.asarray(W2, dtype=np.float32)
    b1 = np.asarray(b1, dtype=np.float32)
    b2 = np.asarray(b2, dtype=np.float32)
    assert np.abs(b1).max() == 0 and np.abs(b2).max() == 0, \
        "nonzero biases not implemented"
    col = np.asarray(edge_index[0]).astype(np.int64)
    row = np.asarray(edge_index[1]).astype(np.int64)

    n, h = emb.shape
    assert h == H
    E = col.shape[0]
    n_pad = ((n + P - 1) // P) * P

    sched, order, core_of, slot_of, caps, R = _deal(col, row)
    col_s, row_s = col[order], row[order]
    half_s = (row_s >= HALF).astype(np.int64)
    gslot = slot_of + half_s * caps[0]           # global slot in [0, S)
    S = caps[0] + caps[1]

    codes = np.full((NCORES, S), -1, dtype=np.int32)
    ridx = np.zeros((NCORES, S), dtype=np.int64)
    opos = np.full((NCORES, S), -1, dtype=np.int64)
    for k in range(NCORES):
        m = core_of == k
        gs = gslot[m]
        codes[k, gs] = col_s[m] & 127
        ridx[k, gs] = row_s[m] - half_s[m] * HALF
        opos[k, gs] = order[m]

    # shared per-batch real counts (identical across cores by dealing)
    gathers = []
    for hh in (0, 1):
        base = 0 if hh == 0 else caps[0]
        for pos0 in range(base, base + caps[hh], BATCH):
            cnt = min(R[hh] - (pos0 - base), BATCH)
            if cnt <= 0:
                continue
            n_reg = min(BATCH, ((cnt + 15) // 16) * 16)
            gathers.append((pos0, n_reg, hh))
            # trailing slots after the shared count: idx -1 (trimmed)
            ridx[:, pos0 + n_reg:pos0 + BATCH] = -1

    chunk_scheds = []
    n_lives = []
    for hh in (0, 1):
        cs, nl = _chunk_schedule(sched[hh], R[hh], caps[hh])
        chunk_scheds.append(cs)
        n_lives.append(nl)

    runs_by_chunk, batch_runs, n_runs = _run_layout(
        caps, chunk_scheds, n_lives)
    # slot -> global run id (for the masked per-run one-hot stream)
    run_of_slot = np.full(S, -1, dtype=np.int64)
    for c, rs in enumerate(runs_by_chunk):
        for (b, off, ln, gid) in rs:
            run_of_slot[c * CHUNK + off:c * CHUNK + off + ln] = gid

    key = (n_pad, tuple(caps), tuple(n_lives), tuple(gathers),
           tuple(tuple(sc) for sc in sched))
    if key not in _PROG_CACHE:
        _PROG_CACHE[key] = _build_program(
            n_pad, caps, chunk_scheds, n_lives, gathers)
    nc = _PROG_CACHE[key]

    embT = np.zeros((P, n_pad), dtype=BF16)
    embT[:, :n] = emb.T.astype(BF16)
    w1t = W1.T.astype(BF16)
    w2t = W2.T.astype(BF16)

    slot_idx = np.arange(S)
    in_maps = []
    for k in range(NCORES):
        oh = np.zeros((P, n_runs * CHUNK), dtype=FP8)
        v = (codes[k] >= 0) & (run_of_slot >= 0)
        oh[codes[k][v],
           run_of_slot[v] * CHUNK + (slot_idx[v] % CHUNK)] = 1
        in_maps.append({
            "embT": embT, "w1t": w1t, "w2t": w2t,
            "oh": oh,
            "ridx": _wrap_idx(ridx[k]),
        })

    res = run_bass_kernel_spmd(nc, in_maps, core_ids=list(range(NCORES)))
    LAST_RESULTS = res

    tbl = np.asarray(res.results[0]["tbl_dram"], dtype=np.float32)
    nrm = np.maximum(np.sqrt((tbl * tbl).sum(axis=1)), 1e-8)[:n]

    out = np.empty(E, dtype=np.float32)
    for k in range(NCORES):
        r = res.results[k]
        d = np.asarray(r["out"], dtype=np.float32)   # [128, n_chunks]
        dots = d.T.reshape(-1)                       # slot-major
        valid = opos[k] >= 0
        out[opos[k][valid]] = dots[valid]
    out /= nrm[col] * nrm[row]
    return out


# revision 6
# speedup vs baseline: 3.8519x; 1.0607x over previous
"""Trainium2 Bass kernel for nn_ExtractorMLP (GNN edge cosine-similarity).

Math:  out[e] = cos_sim(mlp(emb[col[e]]), mlp(emb[row[e]]))
where  mlp(x) = elu(x @ W1.T + b1) @ W2.T + b2   (b1 = b2 = 0 here)

v8 strategy (edge-data-parallel across 8 cores, one SPMD program):
  * Host deals edges so all 8 cores share ONE static schedule: edges are
    bucketed by row-half (int16 gather-index limit), sorted by col-block,
    and each (half, col-block) group is dealt round-robin across the 8
    cores.  Every core holds ceil(n_hb/8) edges of block b -- per-block
    run lengths are IDENTICAL across cores, so the whole phase-2 program
    is compile-time static (no ring/window machinery, no dynamic DMAs).
  * Phase 1 (per node, replicated): unnormalized MLP table written BOTH
    to a persistent SBUF tile (node-major blocks; matmul rhs source) and
    to DRAM (gather source + host-side norm computation).
  * Phase 2 (edges, sharded), EDGE-MAJOR layout (partition = edge):
      - row side (f2): SWDGE dma_gather WITHOUT transpose (edge j ->
        partition j%128, its 256B feature row along the free axis),
        4096 edges per instruction, round-robined over all 4 SWDGE
        queues.  Non-transpose gathers do not touch the shared xbar, so
        concurrent queues are safe (transpose gathers are NOT).
      - col side (f1): per 128-edge chunk, variable-length run matmuls:
        lhsT = fp8 one-hot slice [128 nodes x len] (the edges of one
        col-block), rhs = SBUF table block [128 nodes x 128 feats],
        out = PSUM [len edges x 128 feats] at the chunk's partition
        offset.
      - dot: DVE multiply f1(PSUM) * f2(SBUF) per 512-slot PSUM bank,
        then a segmented tensor_reduce (axis=X) -> [128, 4] dots.
  * Host: out[e] = dots[e] / (n[col] * n[row]), n = max(sqrt(ssq), eps),
    ssq computed on host from the returned bf16 table.
"""

import math

import numpy as np
import ml_dtypes

BF16 = ml_dtypes.bfloat16
FP8 = ml_dtypes.float8_e4m3

H = 128            # feature dim
P = 128            # partitions
CHUNK = 128        # edges per PSUM chunk (partition dim)
BANK = 1024        # DVE op granularity: 2 PSUM banks = 8 chunks
BATCH = 2048       # edges per dma_gather instruction / f2 tile (2 groups)
HALF = 32768       # int16 index limit: row-bucket split
NCORES = 8
ST_W = 512         # phase-1 supertile width (nodes)
NQ = 4             # SWDGE queues for gathers

_PROG_CACHE: dict = {}
LAST_RESULTS = None


# --------------------------------------------------------------------------
# host-side dealing / packing
# --------------------------------------------------------------------------

def _deal(col, row):
    """Deal edges to cores so every core shares the same run schedule.

    Returns:
      sched: per bucket h, list of (block, s_hb) in block order
      order: edge permutation (sorted by (half, block))
      core_of, slot_of: per sorted edge, its core and bucket-local slot
      caps: [2] padded bucket capacities (multiples of BATCH)
      R: [2] shared per-core slot counts (real + intra-block pads)
    """
    E = col.shape[0]
    hb = (row >= HALF).astype(np.int64) * 512 + (col >> 7)
    # row-sort within groups -> gather addresses ascend within runs
    order = np.lexsort((row, hb))
    hb_s = hb[order]
    bounds = np.flatnonzero(np.diff(hb_s)) + 1
    starts = np.concatenate([[0], bounds])
    ends = np.concatenate([bounds, [E]])
    gkeys = hb_s[starts]
    sizes = ends - starts

    s_g = (sizes + NCORES - 1) // NCORES          # shared slots per group
    ghalf = gkeys >> 9
    base = np.zeros(len(gkeys), dtype=np.int64)
    R = [0, 0]
    for h in (0, 1):
        m = ghalf == h
        cs = np.concatenate([[0], np.cumsum(s_g[m])])
        base[m] = cs[:-1]
        R[h] = int(cs[-1])

    j = np.arange(E) - np.repeat(starts, sizes)
    rot = np.zeros(len(gkeys), dtype=np.int64)
    r = 0
    rem = sizes % NCORES
    for gi in range(len(gkeys)):
        rot[gi] = r
        r = (r + int(rem[gi])) % NCORES
    rot_e = np.repeat(rot, sizes)
    core_of = ((j + rot_e) % NCORES).astype(np.int64)
    slot_of = np.repeat(base, sizes) + j // NCORES

    sched = [[], []]
    for gi in range(len(gkeys)):
        sched[int(ghalf[gi])].append((int(gkeys[gi] & 511), int(s_g[gi])))

    caps = [((R[h] + BATCH - 1) // BATCH) * BATCH if R[h] else 0
            for h in (0, 1)]
    return sched, order, core_of, slot_of, caps, R


def _chunk_schedule(sched_h, R_h, cap_h):
    """Split a bucket's run list into 128-edge chunks.

    chunks[c] = [(block, part_off, len)]; chunks past the live count
    have no runs (skipped entirely -- their dots are garbage the host
    ignores).  Each run becomes one full-width accumulate matmul whose
    lhsT is a per-run masked one-hot (PE output partition offsets must
    be quadrant-aligned, so runs cannot slice the partition dim).
    """
    n_chunks = cap_h // CHUNK
    n_live = (R_h + CHUNK - 1) // CHUNK
    chunks = [[] for _ in range(n_chunks)]
    pos = 0
    for (b, s) in sched_h:
        while s > 0:
            c, off = divmod(pos, CHUNK)
            take = min(s, CHUNK - off)
            chunks[c].append((b, off, take))
            pos += take
            s -= take
    return chunks, n_live


# --------------------------------------------------------------------------
# device program
# --------------------------------------------------------------------------

def _run_layout(caps, chunk_scheds, n_lives):
    """Global run numbering in chunk order + per-batch run ranges.

    Returns (runs_by_chunk, batch_runs, n_runs) where runs_by_chunk[c] =
    [(block, run_gid)], batch_runs[bt] = (r0, r1).
    """
    S = caps[0] + caps[1]
    n_chunks = S // CHUNK
    n_lo_chunks = caps[0] // CHUNK
    runs_by_chunk = []
    gid = 0
    for c in range(n_chunks):
        h, lc = (0, c) if c < n_lo_chunks else (1, c - n_lo_chunks)
        rs = []
        if lc < n_lives[h]:
            for (b, off, ln) in chunk_scheds[h][lc]:
                rs.append((b, off, ln, gid))
                gid += 1
        runs_by_chunk.append(rs)
    n_runs = gid
    cpb = BATCH // CHUNK
    batch_runs = []
    for bt in range(S // BATCH):
        gids = [r[3] for c in range(bt * cpb, (bt + 1) * cpb)
                for r in runs_by_chunk[c]]
        if gids:
            batch_runs.append((min(gids), max(gids) + 1))
        else:
            batch_runs.append((0, 0))
    return runs_by_chunk, batch_runs, n_runs


def _build_program(n_pad, caps, chunk_scheds, n_lives, gathers):
    import concourse.bacc as bacc
    import concourse.mybir as mybir
    import concourse.tile as tile
    from concourse import library_config
    from contextlib import ExitStack

    f32 = mybir.dt.float32
    bf16 = mybir.dt.bfloat16
    fp8 = mybir.dt.float8e4
    i16 = mybir.dt.int16
    Alu = mybir.AluOpType
    Act = mybir.ActivationFunctionType
    Axis = mybir.AxisListType

    S = caps[0] + caps[1]
    n_batches = S // BATCH
    n_chunks = S // CHUNK
    n_blocks = n_pad // H
    half = min(HALF, n_pad)

    runs_by_chunk, batch_runs, n_runs = _run_layout(
        caps, chunk_scheds, n_lives)
    max_batch_runs = max(r1 - r0 for (r0, r1) in batch_runs)

    nc = bacc.Bacc("TRN2", target_bir_lowering=False, debug=False,
                   num_devices=NCORES, num_swdge_queues=NQ)

    embT = nc.dram_tensor("embT", [P, n_pad], bf16, kind="ExternalInput")
    w1t_d = nc.dram_tensor("w1t", [H, H], bf16, kind="ExternalInput")
    w2t_d = nc.dram_tensor("w2t", [H, H], bf16, kind="ExternalInput")
    oh_d = nc.dram_tensor("oh", [P, n_runs * CHUNK], fp8,
                          kind="ExternalInput")
    ridx_d = nc.dram_tensor("ridx", [P, S // 16], i16, kind="ExternalInput")
    tbl_d = nc.dram_tensor("tbl_dram", [n_pad, H], bf16, kind="ExternalOutput")
    out_d = nc.dram_tensor("out", [P, n_chunks], f32, kind="ExternalOutput")

    with ExitStack() as ctx:
        tc = ctx.enter_context(tile.TileContext(nc))
        const = ctx.enter_context(tc.tile_pool(name="const", bufs=1))
        p1 = ctx.enter_context(tc.tile_pool(name="p1", bufs=3))
        poh = ctx.enter_context(tc.tile_pool(name="poh", bufs=3))
        # one rxt tile per batch: SWDGE gathers on different queues can
        # retire out of order, so WAR reuse of idx tiles is unsafe
        pridx = ctx.enter_context(tc.tile_pool(name="pridx",
                                               bufs=max(n_batches, 2)))
        pf2 = ctx.enter_context(tc.tile_pool(name="pf2", bufs=8))
        pprod = ctx.enter_context(tc.tile_pool(name="pprod", bufs=4))
        psA = ctx.enter_context(tc.tile_pool(name="psA", bufs=2, space="PSUM"))
        psB = ctx.enter_context(tc.tile_pool(name="psB", bufs=4, space="PSUM"))

        nc.gpsimd.load_library(library_config.mlp)

        # --- constants / persistent tiles ---
        w1t = const.tile([H, H], bf16, tag="w1t")
        w2t = const.tile([H, H], bf16, tag="w2t")
        tblsb = const.tile([P, n_blocks * H], bf16, tag="tblsb")
        dots = const.tile([P, n_chunks], f32, tag="dots")
        nc.sync.dma_start(out=w1t[:], in_=w1t_d[:])
        nc.sync.dma_start(out=w2t[:], in_=w2t_d[:])

        # --- phase 1: MLP table (unnormalized) -> SBUF + DRAM ---
        n0 = 0
        st = 0
        while n0 < n_pad:
            w = min(ST_W, n_pad - n0)
            nb = w // H
            xt = p1.tile([P, ST_W], bf16, tag="xt", name="xt")[:, :w]
            nc.sync.dma_start(out=xt, in_=embT[:, n0:n0 + w])
            ph1 = psA.tile([P, BANK], f32, tag="a", name="ph1")[:, :w]
            nc.tensor.matmul(ph1, lhsT=w1t[:], rhs=xt, start=True, stop=True)
            # elu(x) = max(exp(min(x, 0)) - 1, x)
            u_t = p1.tile([P, ST_W], bf16, tag="u", name="u")[:, :w]
            nc.scalar.activation(u_t, ph1, Act.Relu, scale=-1.0)
            e_t = p1.tile([P, ST_W], bf16, tag="e", name="e")[:, :w]
            nc.scalar.activation(e_t, u_t, Act.Exp, scale=-1.0)
            h1_t = p1.tile([P, ST_W], bf16, tag="h1", name="h1")[:, :w]
            nc.vector.scalar_tensor_tensor(
                h1_t, in0=e_t, scalar=-1.0, in1=ph1,
                op0=Alu.add, op1=Alu.max)
            pg = psB.tile([P, ST_W], f32, tag="b", name="pg")[:, :w]
            for b in range(nb):
                nc.tensor.matmul(pg[:, b * H:(b + 1) * H],
                                 lhsT=h1_t[:, b * H:(b + 1) * H],
                                 rhs=w2t[:], start=True, stop=True)
            tb_t = tblsb[:, n0:n0 + w]
            # alternate the PSUM->SBUF copy between scalar and vector
            if st % 2 == 0:
                nc.scalar.activation(tb_t, pg, Act.Copy)
            else:
                nc.vector.tensor_copy(out=tb_t, in_=pg)
            nc.sync.dma_start(
                out=tbl_d[n0:n0 + w, :].rearrange("(s p) f -> p s f", p=P),
                in_=tb_t.rearrange("p (s f) -> p s f", f=H))
            n0 += w
            st += 1

        # --- phase 2 (edge-major) ---
        halves = (tbl_d[0:half, :], tbl_d[half:n_pad, :])
        n_lo_chunks = caps[0] // CHUNK

        def _loc(c):
            return (0, c) if c < n_lo_chunks else (1, c - n_lo_chunks)

        live = [(_loc(c)[1] < n_lives[_loc(c)[0]]) for c in range(n_chunks)]

        g_by_batch = {}
        for (pos0, n_reg, hf) in gathers:
            assert pos0 % BATCH == 0 and 0 < n_reg <= BATCH
            g_by_batch[pos0 // BATCH] = (pos0, n_reg, hf)

        # preload ALL gather index tiles upfront: lazy loads starve
        # behind in-flight gather descriptor bursts and serialize the
        # gather stream
        rx_tiles = {}
        for (pos0, n_reg, hf) in gathers:
            bt = pos0 // BATCH
            rxt = pridx.tile([P, BATCH // 16], i16, tag="rx",
                             name=f"rx{bt}")
            rx_tiles[bt] = rxt
            nc.scalar.dma_start(
                out=rxt[:],
                in_=ridx_d[:, pos0 // 16:(pos0 + BATCH) // 16])

        f2_tiles = {}
        oh_tiles = {}
        oh_base = {}
        qn = 0
        for c0 in range(0, n_chunks, BANK // CHUNK):
            if (c0 * CHUNK) % BATCH == 0:
                bt = c0 * CHUNK // BATCH
                if bt in g_by_batch:
                    (pos0, n_reg, hf) = g_by_batch[bt]
                    f2t = pf2.tile([P, BATCH], bf16, tag="f2",
                                   name=f"f2_{bt}")
                    f2_tiles[bt] = f2t
                    f2g = f2t[:, 0:BATCH].rearrange("p (a t) -> p a t", t=H)
                    nc.gpsimd.dma_gather(
                        f2g, halves[hf], rx_tiles[bt][:],
                        BATCH, n_reg, H,
                        transpose=False, single_packet=False,
                        queue_num=qn % NQ)
                    qn += 1
                    (r0, r1) = batch_runs[bt]
                    if r1 > r0:
                        oht = poh.tile([P, max_batch_runs * CHUNK], fp8,
                                       tag="ohd", name=f"ohb{bt}")
                        oh_tiles[bt] = oht
                        oh_base[bt] = r0
                        nc.scalar.dma_start(
                            out=oht[:, :(r1 - r0) * CHUNK],
                            in_=oh_d[:, r0 * CHUNK:r1 * CHUNK])
            bank_chunks = [c for c in range(c0, c0 + BANK // CHUNK)
                           if live[c]]
            if not bank_chunks:
                continue
            bt = c0 * CHUNK // BATCH
            off_b = c0 * CHUNK % BATCH          # slot offset within batch
            f1p = psA.tile([P, BANK], f32, tag="a", name=f"f1_{c0}")
            for c in bank_chunks:
                q = c - c0
                rs = runs_by_chunk[c]
                for i, (b, off, ln, gid) in enumerate(rs):
                    o0 = (gid - oh_base[bt]) * CHUNK
                    nc.tensor.matmul(
                        f1p[:, q * H:(q + 1) * H],
                        lhsT=oh_tiles[bt][:, o0:o0 + CHUNK],
                        rhs=tblsb[:, b * H:(b + 1) * H],
                        start=(i == 0), stop=(i == len(rs) - 1))
            prod = pprod.tile([P, BANK], bf16, tag="prod", name=f"pr{c0}")
            nc.vector.tensor_tensor(
                out=prod[:], in0=f1p[:],
                in1=f2_tiles[bt][:, off_b:off_b + BANK], op=Alu.mult)
            nc.vector.tensor_reduce(
                out=dots[:, c0:c0 + BANK // CHUNK],
                in_=prod.rearrange("p (q f) -> p q f", f=H),
                axis=Axis.X, op=Alu.add)
        nc.sync.dma_start(out=out_d[:], in_=dots[:])

    nc.compile()
    return nc


# --------------------------------------------------------------------------
# numpy emulation of phase 2 (host-side self-test)
# --------------------------------------------------------------------------

def _emulate_core(table_f32, oh, ridx, caps, chunk_scheds, n_lives, gathers):
    S = caps[0] + caps[1]
    n_chunks = S // CHUNK
    dots = np.zeros(S, dtype=np.float32)
    f2_full = np.zeros((S, H), dtype=np.float32)
    for (pos0, n_reg, hf) in gathers:
        base = hf * HALF
        idx = ridx[pos0:pos0 + n_reg].astype(np.int64)
        idx = np.where(idx >= 0, idx + base, 0)
        f2_full[pos0:pos0 + n_reg, :] = table_f32[idx, :]
    n_lo_chunks = caps[0] // CHUNK
    for c in range(n_chunks):
        h, lc = (0, c) if c < n_lo_chunks else (1, c - n_lo_chunks)
        if lc >= n_lives[h]:
            continue
        f1 = np.zeros((CHUNK, H), dtype=np.float32)
        ohc = oh[:, c * CHUNK:(c + 1) * CHUNK].astype(np.float32)
        for (b, off, ln) in chunk_scheds[h][lc]:
            f1[off:off + ln, :] = (
                ohc[:, off:off + ln].T @ table_f32[b * P:(b + 1) * P, :])
        prod = (f1 * f2_full[c * CHUNK:(c + 1) * CHUNK, :]
                ).astype(BF16).astype(np.float32)
        dots[c * CHUNK:(c + 1) * CHUNK] = prod.sum(axis=1)
    return dots


# --------------------------------------------------------------------------
# entry point
# --------------------------------------------------------------------------

def _ensure_ntff_hook():
    import sys
    import types
    try:
        import antenv.axon_hooks  # noqa: F401
        return
    except ImportError:
        pass
    try:
        import antenv
        from trn_agent_boot.trn_boot import _ntff_profile_via_ctypes
        mod = types.ModuleType("antenv.axon_hooks")
        mod._hook = _ntff_profile_via_ctypes("/opt/axon/libaxon_pjrt.so")
        mod.get_axon_ntff_profile_hook = lambda: mod._hook
        mod.set_axon_ntff_profile_hook = lambda h: setattr(mod, "_hook", h)
        sys.modules["antenv.axon_hooks"] = mod
        antenv.axon_hooks = mod
    except Exception:
        pass


def _wrap_idx(idx):
    """[S] int -> [128, S/16] int16 wrapped (16 partitions, replicated 8x)."""
    w = idx.reshape(-1, 16).T.astype(np.int16)
    return np.tile(w, (8, 1))


def kernel(emb, edge_index, W1, b1, W2, b2):
    global LAST_RESULTS
    from concourse.bass_utils import run_bass_kernel_spmd
    _ensure_ntff_hook()

    emb = np.asarray(emb, dtype=np.float32)
    W1 = np.asarray(W1, dtype=np.float32)
    W2 = np.asarray(W2, dtype=np.float32)
    b1 = np.asarray(b1, dtype=np.float32)
    b2 = np.asarray(b2, dtype=np.float32)
    assert np.abs(b1).max() == 0 and np.abs(b2).max() == 0, \
        "nonzero biases not implemented"
    col = np.asarray(edge_index[0]).astype(np.int64)
    row = np.asarray(edge_index[1]).astype(np.int64)

    n, h = emb.shape
    assert h == H
    E = col.shape[0]
    n_pad = ((n + P - 1) // P) * P

    sched, order, core_of, slot_of, caps, R = _deal(col, row)
    col_s, row_s = col[order], row[order]
    half_s = (row_s >= HALF).astype(np.int64)
    gslot = slot_of + half_s * caps[0]           # global slot in [0, S)
    S = caps[0] + caps[1]

    codes = np.full((NCORES, S), -1, dtype=np.int32)
    ridx = np.zeros((NCORES, S), dtype=np.int64)
    opos = np.full((NCORES, S), -1, dtype=np.int64)
    for k in range(NCORES):
        m = core_of == k
        gs = gslot[m]
        codes[k, gs] = col_s[m] & 127
        ridx[k, gs] = row_s[m] - half_s[m] * HALF
        opos[k, gs] = order[m]

    # shared per-batch real counts (identical across cores by dealing)
    gathers = []
    for hh in (0, 1):
        base = 0 if hh == 0 else caps[0]
        for pos0 in range(base, base + caps[hh], BATCH):
            cnt = min(R[hh] - (pos0 - base), BATCH)
            if cnt <= 0:
                continue
            n_reg = min(BATCH, ((cnt + 15) // 16) * 16)
            gathers.append((pos0, n_reg, hh))
            # trailing slots after the shared count: idx -1 (trimmed)
            ridx[:, pos0 + n_reg:pos0 + BATCH] = -1

    chunk_scheds = []
    n_lives = []
    for hh in (0, 1):
        cs, nl = _chunk_schedule(sched[hh], R[hh], caps[hh])
        chunk_scheds.append(cs)
        n_lives.append(nl)

    runs_by_chunk, batch_runs, n_runs = _run_layout(
        caps, chunk_scheds, n_lives)
    # slot -> global run id (for the masked per-run one-hot stream)
    run_of_slot = np.full(S, -1, dtype=np.int64)
    for c, rs in enumerate(runs_by_chunk):
        for (b, off, ln, gid) in rs:
            run_of_slot[c * CHUNK + off:c * CHUNK + off + ln] = gid

    key = (n_pad, tuple(caps), tuple(n_lives), tuple(gathers),
           tuple(tuple(sc) for sc in sched))
    if key not in _PROG_CACHE:
        _PROG_CACHE[key] = _build_program(
            n_pad, caps, chunk_scheds, n_lives, gathers)
    nc = _PROG_CACHE[key]

    embT = np.zeros((P, n_pad), dtype=BF16)
    embT[:, :n] = emb.T.astype(BF16)
    w1t = W1.T.astype(BF16)
    w2t = W2.T.astype(BF16)

    slot_idx = np.arange(S)
    in_maps = []
    for k in range(NCORES):
        oh = np.zeros((P, n_runs * CHUNK), dtype=FP8)
        v = (codes[k] >= 0) & (run_of_slot >= 0)
        oh[codes[k][v],
           run_of_slot[v] * CHUNK + (slot_idx[v] % CHUNK)] = 1
        in_maps.append({
            "embT": embT, "w1t": w1t, "w2t": w2t,
            "oh": oh,
            "ridx": _wrap_idx(ridx[k]),
        })

    res = run_bass_kernel_spmd(nc, in_maps, core_ids=list(range(NCORES)))
    LAST_RESULTS = res

    tbl = np.asarray(res.results[0]["tbl_dram"], dtype=np.float32)
    nrm = np.maximum(np.sqrt((tbl * tbl).sum(axis=1)), 1e-8)[:n]

    out = np.empty(E, dtype=np.float32)
    for k in range(NCORES):
        r = res.results[k]
        d = np.asarray(r["out"], dtype=np.float32)   # [128, n_chunks]
        dots = d.T.reshape(-1)                       # slot-major
        valid = opos[k] >= 0
        out[opos[k][valid]] = dots[valid]
    out /= nrm[col] * nrm[row]
    return out
